# revision 63
# baseline (speedup 1.0000x reference)
"""BERT self-attention on 8 Trainium2 NeuronCores.

Sharding: data-parallel over batch (4 cores per batch element) x
tensor-parallel over heads (4 heads per core). Q/K/V projections are
column-sharded, the output projection is row-sharded; each core returns a
partial [S, D] output which the host sums (+ b_o).

Per-core math (batch b, heads hs = 4 heads, cols = 256 feature slice):
  QT = Wq_sl.T @ X_b.T        [256, 2048]   (bias via ScalarE Identity)
  KT = Wk_sl.T @ X_b.T        [256, 2048]   (m=0 half up front, m=1 in bg)
  V  = X_b @ Wv_sl            [2048, 256] -> V' = [V_h*mask | mask] per
                              head (b_v is folded into normalize instead:
                              exact since sum(probs) = 1 per query)
  per head h, q-block qb (512 wide):
    S^T tile [128k, 512q] = K_h @ Q_h^T slices  (PSUM)
    expS = exp(S^T / 8)                          (ACT, PSUM->SBUF)
    O'   = V'_h.T @ expS   accumulated over 16 k-tiles -> [65, 512]
           rows 0:64 = unnormalized O_h^T, row 64 = softmax denominator
  normalize: O^T = O^T * (1/den) + b_v (fast approx reciprocal; den
  broadcast across partitions via a stride-0 DMA read from DRAM staging)
  Y_partial = O^T.T @ Wo_sl   [2048, 1024]

Schedule: the exp stream on ScalarE is the attention-phase pacer, so
attention starts right after QT + KT(m=0); the KT m=1 chunks and all V
s-tiles are drained one-per-score-group as background PE work inside the
attention loop (sharing one PSUM bank that is later handed to the Y
projection), and PV groups are drained from a deep pending queue that
keeps the PE dense between exp handoffs. Bias adds run on ScalarE
(Identity+bias), the V'-mask fold is a single stride-0-broadcast
tensor-tensor multiply, and softmax division uses the single-pass
reciprocal_approx_fast.

Matmuls accumulate in fp32 PSUM; the QK/QT path and V'/expS run in bf16
(full PE rate + fast weight load), the output projection in float32r
(FP22). K tiles are zero-padded to the full 128 contraction rows because
the PE clock gate (HAM) only unthrottles for full-row matmuls.
"""

import sys

for _p in ("/root/.axon_site/_ro/trn_rl_repo", "/opt/trn_rl_repo"):
    if _p not in sys.path:
        sys.path.append(_p)

import numpy as np
import ml_dtypes

BF16 = ml_dtypes.bfloat16

B, S, D, H, DH = 2, 2048, 1024, 16, 64
P = 128
NCORES = 8
HPC = 4              # heads per core
CW = HPC * DH        # 256: per-core feature slice width
DK = D // P          # 8 k-tiles over the model dim
SP = S // P          # 16 s-tiles
NB = 4               # 512-wide blocks over S
NW = S // NB         # 512
G = 3                # exp kt-group size (PSUM banks per stage tile)

_STATE = {}


def _build_nc():
    import concourse.bacc as bacc
    import concourse.tile as tile
    from concourse import mybir

    f32 = mybir.dt.float32
    f32r = mybir.dt.float32r
    bf16 = mybir.dt.bfloat16
    Exp = mybir.ActivationFunctionType.Exp
    ActId = mybir.ActivationFunctionType.Identity

    nc = bacc.Bacc(None, target_bir_lowering=False, debug=False)

    with tile.TileContext(nc) as tc:
        with tc.tile_pool(name="dram", bufs=1, space="DRAM") as dram:
            xt = dram.tile([D, S], bf16, kind="ExternalInput", name="xt", uniquify=False)
            wq = dram.tile([P, DK, CW], bf16, kind="ExternalInput", name="wq", uniquify=False)
            wk = dram.tile([P, DK, CW], bf16, kind="ExternalInput", name="wk", uniquify=False)
            wv = dram.tile([P, DK, CW], bf16, kind="ExternalInput", name="wv", uniquify=False)
            wo = dram.tile([P, CW // P, D], f32, kind="ExternalInput", name="wo", uniquify=False)
            bq = dram.tile([P, CW // P], f32, kind="ExternalInput", name="bq", uniquify=False)
            bk = dram.tile([P, CW // P], f32, kind="ExternalInput", name="bk", uniquify=False)
            bv = dram.tile([P, CW // P], f32, kind="ExternalInput", name="bv", uniquify=False)
            vmask = dram.tile([P, SP, HPC], f32, kind="ExternalInput", name="vmask", uniquify=False)
            y = dram.tile([S, D], f32, kind="ExternalOutput", name="y", uniquify=False)
            dden = dram.tile([HPC * NB, NW], f32, name="dden")

            import concourse.bass as bass

            consts_cm = tc.tile_pool(name="consts", bufs=1)
            consts = consts_cm.__enter__()
            xt_sb = consts.tile([P, DK, S], bf16, name="xt_sb")
            wq_sb = consts.tile([P, DK, CW], bf16, name="wq_sb")
            wk_sb = consts.tile([P, DK, CW], bf16, name="wk_sb")
            wv_sb = consts.tile([P, DK, CW], bf16, name="wv_sb")
            wo_sb = consts.tile([P, CW // P, D], f32r, name="wo_sb")
            bq_sb = consts.tile([P, CW // P], f32, name="bq_sb")
            bk_sb = consts.tile([P, CW // P], f32, name="bk_sb")
            bv_sb = consts.tile([P, CW // P], f32, name="bv_sb")
            vmask_sb = consts.tile([P, SP, HPC], f32, name="vmask_sb")
            qt_sb = consts.tile([P, CW // P, S], bf16, name="qt_sb")
            kz_sb = consts.tile([P, HPC, SP, P], bf16, name="kz_sb")
            zsrc = consts.tile([P, NW], bf16, name="zsrc")
            ones_sb = consts.tile([P, DH], f32, name="ones_sb")
            vp_sb = consts.tile([P, SP, HPC, DH + 1], bf16, name="vp_sb")
            ot_sb = consts.tile([P, CW // P, S], f32r, name="ot_sb")

            # ---- input DMAs (small tensors first so the first matmuls
            # aren't queued behind the 8MB of X^T) ----
            nc.sync.dma_start(out=wq_sb[:], in_=wq[:])
            nc.sync.dma_start(out=bq_sb[:], in_=bq[:])
            for k in range(DK):
                for hlf in range(2):
                    hs_ = slice(hlf * (S // 2), (hlf + 1) * (S // 2))
                    nc.sync.dma_start(out=xt_sb[:, k, hs_],
                                      in_=xt[k * P:(k + 1) * P, hs_])
            nc.sync.dma_start(out=wk_sb[:], in_=wk[:])
            nc.sync.dma_start(out=bk_sb[:], in_=bk[:])
            nc.sync.dma_start(out=wv_sb[:], in_=wv[:])
            nc.sync.dma_start(out=wo_sb[:], in_=wo[:].bitcast(f32r))
            nc.sync.dma_start(out=bv_sb[:], in_=bv[:])
            nc.sync.dma_start(out=vmask_sb[:], in_=vmask[:])

            # zero-fill kz (stride-0 free-dim broadcast of a zeroed tile);
            # each head's K^T occupies its natural 64 partition rows, the
            # other 64 rows stay zero so the scores matmul contracts over
            # the full 128 partitions (HAM keeps the PE clock warm only
            # for full-row matmuls)
            nc.vector.memset(zsrc[:], 0.0)
            nc.vector.memset(ones_sb[:], 1.0)
            zview = zsrc[:]
            zbc = bass.AP(
                tensor=zview.tensor, offset=zview.offset,
                ap=[list(zview.ap[0]), [0, HPC * SP * P // NW]] + list(zview.ap[1:]))
            nc.vector.tensor_copy(out=kz_sb[:], in_=zbc)

            # warm-up burst: ~12 dummy matmuls on zeros while the X^T DMA
            # streams in, so the PE clock gate (HAM) is already at 8/8 when
            # the real projection matmuls start
            with tc.tile_pool(name="warm_psum", bufs=1, space="PSUM") as warm_psum:
                wps = warm_psum.tile([P, NW], f32, name="wps")
                for _ in range(12):
                    nc.tensor.matmul(wps[:], zsrc[:, 0:P], zsrc[:],
                                     start=True, stop=True)

            # ---- projections ----
            # QT first, k-outer (8 live PSUM accumulators) so matmuls pace
            # with the X^T DMA stream instead of waiting for all of it
            with tc.tile_pool(name="proj_psum", bufs=8, space="PSUM") as proj_psum:
                psqs = {}
                for m in range(CW // P):
                    for nb in range(NB):
                        psqs[(m, nb)] = proj_psum.tile([P, NW], f32, name="psq", tag="pp")
                for k in range(DK):
                    for m in range(CW // P):
                        for nb in range(NB):
                            nc.tensor.matmul(
                                psqs[(m, nb)][:], wq_sb[:, k, m * P:(m + 1) * P],
                                xt_sb[:, k, nb * NW:(nb + 1) * NW],
                                start=(k == 0), stop=(k == DK - 1))
                # per-(m,nb) bias adds split across ScalarE and DVE: each
                # 1-bank psq tile frees as soon as its own add retires, so
                # the first KT chunk's bank reuse isn't gated on the whole
                # QT readout
                for nb in range(NB):
                    nc.scalar.activation(
                        out=qt_sb[:, 0, nb * NW:(nb + 1) * NW],
                        in_=psqs[(0, nb)][:], func=ActId,
                        bias=bq_sb[:, 0:1])
                    nc.vector.tensor_scalar_add(
                        out=qt_sb[:, 1, nb * NW:(nb + 1) * NW],
                        in0=psqs[(1, nb)][:], scalar1=bq_sb[:, 1:2])

            # K projection: only the m=0 half (heads 0/1) before attention —
            # the m=1 half and the whole V projection are emitted as
            # background work inside the attention loop, filling the PE
            # while the exp stream (the attention-phase pacer) runs.
            def emit_kt_chunk(kpool, m, nb, on_act=True):
                psk = kpool.tile([P, NW], f32, name="bgp")
                for k in range(DK):
                    nc.tensor.matmul(
                        psk[:], wk_sb[:, k, m * P:(m + 1) * P],
                        xt_sb[:, k, nb * NW:(nb + 1) * NW],
                        start=(k == 0), stop=(k == DK - 1))
                # bias-scatter on ScalarE pre-attention (it is idle then),
                # but on DVE for background chunks inside the attention
                # loop — extra ACT work there delays the exp stream, the
                # attention-phase pacer
                if on_act:
                    nc.scalar.activation(
                        out=kz_sb[0:DH, 2 * m, nb * 4:(nb + 1) * 4, :],
                        in_=psk[0:DH, :].rearrange("p (a b) -> p a b", a=4),
                        func=ActId, bias=bk_sb[0:DH, m:m + 1])
                    nc.scalar.activation(
                        out=kz_sb[DH:P, 2 * m + 1, nb * 4:(nb + 1) * 4, :],
                        in_=psk[DH:P, :].rearrange("p (a b) -> p a b", a=4),
                        func=ActId, bias=bk_sb[DH:P, m:m + 1])
                else:
                    nc.vector.tensor_scalar_add(
                        out=kz_sb[0:DH, 2 * m, nb * 4:(nb + 1) * 4, :],
                        in0=psk[0:DH, :].rearrange("p (a b) -> p a b", a=4),
                        scalar1=bk_sb[0:DH, m:m + 1])
                    nc.vector.tensor_scalar_add(
                        out=kz_sb[DH:P, 2 * m + 1, nb * 4:(nb + 1) * 4, :],
                        in0=psk[DH:P, :].rearrange("p (a b) -> p a b", a=4),
                        scalar1=bk_sb[DH:P, m:m + 1])

            def emit_v_st(vpool, st):
                bgt = vpool.tile([P, NW], f32, name="bgp")
                psv = bgt[:, 0:CW]
                for k in range(DK):
                    nc.tensor.matmul(
                        psv, xt_sb[:, k, st * P:(st + 1) * P],
                        wv_sb[:, k, :],
                        start=(k == 0), stop=(k == DK - 1))
                # mask-fold + scatter into [h, 65]-strided V' slots, one
                # tensor_tensor with a stride-0 d-broadcast of the mask.
                # b_v is NOT added here: since sum(probs)=1 per query, the
                # V bias commutes with attention and is added during
                # normalize instead.
                vm = vmask_sb[:, st, :]
                vm_bc = bass.AP(
                    tensor=vm.tensor, offset=vm.offset,
                    ap=list(vm.ap) + [[0, DH]])
                nc.vector.tensor_mul(
                    out=vp_sb[:, st, :, 0:DH],
                    in0=psv.rearrange("p (h d) -> p h d", h=HPC),
                    in1=vm_bc)
                # ones column times mask == mask itself
                nc.vector.tensor_copy(
                    out=vp_sb[:, st, :, DH:DH + 1], in_=vmask_sb[:, st, :])

            with tc.tile_pool(name="kproj_psum", bufs=2, space="PSUM") as kproj_psum:
                for nb in range(NB):
                    emit_kt_chunk(kproj_psum, 0, nb)

            # ---- attention + inline normalize/output projection ----
            # qb-outer so each q-block's normalize + Y projection can be
            # emitted one head-iteration behind, overlapping the next
            # block's attention and keeping the PE dense (and HAM-warm)
            kt_groups = [list(range(g * G, min(g * G + G, SP))) for g in range((SP + G - 1) // G)]

            sb_pools_cm = [
                tc.tile_pool(name="exps_pool", bufs=15),
                tc.tile_pool(name="st65_pool", bufs=3),
                tc.tile_pool(name="bcast_pool", bufs=2),
                tc.tile_pool(name="y_pool", bufs=2),
            ]
            exps_pool, st65_pool, bcast_pool, y_pool = [c.__enter__() for c in sb_pools_cm]
            # background-work PSUM bank (KT m=1 chunks, V s-tiles) lives
            # until the bg queue drains, then its bank is handed to y_psum.
            # Pools must close in stack order, so bgpool enters last.
            bgpool_cm = tc.tile_pool(name="bg_psum", bufs=1, space="PSUM")
            y_psum_cm = tc.tile_pool(name="y_psum", bufs=1, space="PSUM")
            y_psum = None
            with tc.tile_pool(name="stage_psum", bufs=2, space="PSUM") as stage_psum, \
                 tc.tile_pool(name="op_psum", bufs=1, space="PSUM") as op_psum:
                bgpool = bgpool_cm.__enter__()

                def emit_pv(p):
                    op_p, h_p, kts_p, ex_p = p
                    for i, kt in enumerate(kts_p):
                        nc.tensor.matmul(
                            op_p[:], vp_sb[:, kt, h_p, :], ex_p[:, i, :],
                            start=(kt == 0), stop=(kt == SP - 1),
                            skip_group_check=True)

                tail_st65 = {}

                def emit_evict(p):
                    op_p, h_p, qb_p = p
                    qs = slice(qb_p * NW, (qb_p + 1) * NW)
                    mt, po = h_p // 2, (h_p % 2) * DH
                    st65 = st65_pool.tile([DH + 1, NW], f32, name="st65")
                    nc.vector.tensor_copy(out=st65[:], in_=op_p[:])
                    nc.sync.dma_start(
                        out=ot_sb[po:po + DH, mt, qs], in_=st65[0:DH, :].bitcast(f32r))
                    nc.sync.dma_start(
                        out=dden[h_p * NB + qb_p, :], in_=st65[DH:DH + 1, :])
                    if qb_p == NB - 1 and h_p >= 2:
                        # the tail normalize broadcasts these rows via a
                        # PE ones-matmul instead of a DRAM roundtrip
                        tail_st65[h_p] = st65

                def emit_normalize(qb_p, mt):
                    qs = slice(qb_p * NW, (qb_p + 1) * NW)
                    bc = bcast_pool.tile([P, NW], f32, name="bc")
                    for half in range(2):
                        hh = 2 * mt + half
                        den_row = dden[hh * NB + qb_p:hh * NB + qb_p + 1, :]
                        den_bcast = bass.AP(
                            tensor=den_row.tensor,
                            offset=den_row.offset,
                            ap=[[0, DH]] + list(den_row.ap[1:]),
                        )
                        nc.sync.dma_start(
                            out=bc[half * DH:(half + 1) * DH, :], in_=den_bcast)
                    nc.vector.reciprocal_approx_fast(out=bc[:], in_=bc[:])
                    # ot = ot * (1/den) + b_v  (b_v folded in here; exact
                    # because sum(probs) = 1 per query)
                    nc.vector.tensor_mul(
                        out=ot_sb[:, mt, qs],
                        in0=ot_sb[:, mt, qs], in1=bc[:])
                    nc.vector.tensor_scalar_add(
                        out=ot_sb[:, mt, qs],
                        in0=ot_sb[:, mt, qs], scalar1=bv_sb[:, mt:mt + 1])

                def emit_y_st(st):
                    yt = y_pool.tile([P, D], f32, name="yt")
                    for n2 in range(2):
                        yps = y_psum.tile([P, NW], f32, name="yps")
                        for k2 in range(CW // P):
                            nc.tensor.matmul(
                                yps[:], ot_sb[:, k2, st * P:(st + 1) * P],
                                wo_sb[:, k2, n2 * NW:(n2 + 1) * NW],
                                start=(k2 == 0), stop=(k2 == CW // P - 1))
                        nc.vector.tensor_copy(
                            out=yt[:, n2 * NW:(n2 + 1) * NW], in_=yps[:])
                    nc.sync.dma_start(out=y[st * P:(st + 1) * P, :], in_=yt[:])

                from collections import deque
                pends = deque()     # (op_ps, h, kts, ex, last, qb)
                evicted = set()
                def drain_one():
                    op_p, h_p, kts_p, ex_p, last, qb_p = pends.popleft()
                    emit_pv((op_p, h_p, kts_p, ex_p))
                    if last:
                        emit_evict((op_p, h_p, qb_p))
                        evicted.add((qb_p, h_p))

                def drain_until(qb_p, h_p):
                    while (qb_p, h_p) not in evicted:
                        drain_one()

                # background PE work drained one item per score-group: the
                # m=1 K chunks first (needed by h>=2), then the V s-tiles
                bg = deque([("kt", nb) for nb in range(NB)]
                           + [("v", st) for st in range(SP)])
                v_drained = 0

                for qb in range(NB):
                    qs = slice(qb * NW, (qb + 1) * NW)
                    for h in range(HPC):
                        mt = h // 2
                        if h == 0 and qb > 0:
                            drain_until(qb - 1, 1)
                            emit_normalize(qb - 1, 0)
                        if h == 1 and qb > 0:
                            drain_until(qb - 1, 3)
                            emit_normalize(qb - 1, 1)
                        op_ps = op_psum.tile([DH + 1, NW], f32, name="op_ps")
                        for gi, kts in enumerate(kt_groups):
                            ng = len(kts)
                            st_ps = stage_psum.tile([P, G, NW], f32, name="st_ps")
                            for i, kt in enumerate(kts):
                                nc.tensor.matmul(
                                    st_ps[:, i, :],
                                    kz_sb[:, h, kt, :],
                                    qt_sb[:, mt, qs],
                                    start=True, stop=True)
                            if bg:
                                kind, idx = bg.popleft()
                                if kind == "kt":
                                    emit_kt_chunk(bgpool, 1, idx, on_act=False)
                                else:
                                    emit_v_st(bgpool, idx)
                                    v_drained += 1
                                # extra PV drain only when its V' s-tiles
                                # are already emitted (read-after-write on
                                # vp_sb must follow program order); when
                                # blocked, burn the slot on a second bg
                                # item so the queue unblocks sooner
                                if len(pends) >= 9:
                                    if not bg or max(pends[0][2]) < v_drained:
                                        drain_one()
                                    else:
                                        kind, idx = bg.popleft()
                                        if kind == "kt":
                                            emit_kt_chunk(bgpool, 1, idx,
                                                          on_act=False)
                                        else:
                                            emit_v_st(bgpool, idx)
                                            v_drained += 1
                                if not bg:
                                    # hand the bg bank to the Y projection
                                    bgpool_cm.__exit__(None, None, None)
                                    y_psum = y_psum_cm.__enter__()
                            else:
                                if pends:
                                    drain_one()
                                if len(pends) >= 8:
                                    drain_one()
                            if qb > 0 and h >= 2 and gi in (1, 3):
                                sts = 4 * (qb - 1) + 2 * (h - 2) + (gi - 1) // 2
                                emit_y_st(sts)
                            ex = exps_pool.tile([P, G, NW], bf16, name="ex")
                            nc.scalar.activation(
                                out=ex[:, 0:ng, :], in_=st_ps[:, 0:ng, :],
                                func=Exp, scale=1.0 / np.sqrt(DH))
                            pends.append((op_ps, h, kts, ex, gi == len(kt_groups) - 1, qb))
                # tail: overlap the last q-block's mt0 normalize and the
                # first half of the mt1 denominator broadcast with the
                # remaining PV drains
                drain_until(NB - 1, 1)
                emit_normalize(NB - 1, 0)
                drain_until(NB - 1, 2)
                # prefetch the h2 half of the last denominator broadcast so
                # only the h3 half remains on the tail critical path
                bct = bcast_pool.tile([P, NW], f32, name="bct")
                r0 = 2 * NB + (NB - 1)          # dden row for (head 2, qb3)
                den_row = dden[r0:r0 + 1, :]
                den_bcast = bass.AP(
                    tensor=den_row.tensor, offset=den_row.offset,
                    ap=[[0, DH]] + list(den_row.ap[1:]))
                nc.sync.dma_start(out=bct[0:DH, :], in_=den_bcast)
                while pends:
                    drain_one()
                if y_psum is not None:
                    y_psum_cm.__exit__(None, None, None)

            # tail: last q-block's remaining normalize + output projection,
            # with the attention PSUM banks freed for deeper Y buffering
            with tc.tile_pool(name="y2_psum", bufs=4, space="PSUM") as y2_psum:
                # last q-block, second head-pair: normalize per 128-wide
                # chunk so each Y st-tile starts as soon as its slice is
                # ready instead of waiting on the full 512-wide mul
                qb_t, mt_t = NB - 1, 1
                r1 = (2 * mt_t + 1) * NB + qb_t  # dden row for (head 3, qb3)
                den_row = dden[r1:r1 + 1, :]
                den_bcast = bass.AP(
                    tensor=den_row.tensor, offset=den_row.offset,
                    ap=[[0, DH]] + list(den_row.ap[1:]))
                nc.sync.dma_start(out=bct[DH:P, :], in_=den_bcast)
                nc.vector.reciprocal_approx_fast(out=bct[:], in_=bct[:])
                for st in range(4 * (NB - 1), 4 * NB):
                    cs = slice((st % 4) * P, (st % 4 + 1) * P)
                    qs = slice(st * P, (st + 1) * P)
                    nc.vector.tensor_mul(
                        out=ot_sb[:, mt_t, qs], in0=ot_sb[:, mt_t, qs],
                        in1=bct[:, cs])
                    nc.vector.tensor_scalar_add(
                        out=ot_sb[:, mt_t, qs], in0=ot_sb[:, mt_t, qs],
                        scalar1=bv_sb[:, mt_t:mt_t + 1])
                    yt = y_pool.tile([P, D], f32, name="yt")
                    for n2 in range(2):
                        yps = y2_psum.tile([P, NW], f32, name="yps")
                        for k2 in range(CW // P):
                            nc.tensor.matmul(
                                yps[:], ot_sb[:, k2, qs],
                                wo_sb[:, k2, n2 * NW:(n2 + 1) * NW],
                                start=(k2 == 0), stop=(k2 == CW // P - 1))
                        # PSUM->SBUF evictions split across ScalarE (idle
                        # after the last exp) and DVE so neither serializes
                        # the tail
                        if n2 == 0:
                            nc.scalar.activation(
                                out=yt[:, 0:NW], in_=yps[:], func=ActId)
                        else:
                            nc.vector.tensor_copy(
                                out=yt[:, NW:D], in_=yps[:])
                    nc.sync.dma_start(out=y[st * P:(st + 1) * P, :], in_=yt[:])

            for c in reversed(sb_pools_cm):
                c.__exit__(None, None, None)
            consts_cm.__exit__(None, None, None)

    nc.compile()
    return nc


def _get_nc():
    if "nc" not in _STATE:
        _STATE["nc"] = _build_nc()
    return _STATE["nc"]


def _make_in_maps(hidden_states, attention_mask, W_q, b_q, W_k, b_k, W_v, b_v, W_o):
    hs = np.asarray(hidden_states, dtype=np.float32)
    mask = np.asarray(attention_mask)
    W_q = np.asarray(W_q, dtype=np.float32)
    W_k = np.asarray(W_k, dtype=np.float32)
    W_v = np.asarray(W_v, dtype=np.float32)
    W_o = np.asarray(W_o, dtype=np.float32)
    b_q = np.asarray(b_q, dtype=np.float32)
    b_k = np.asarray(b_k, dtype=np.float32)
    b_v = np.asarray(b_v, dtype=np.float32)

    in_maps = []
    for c in range(NCORES):
        b, j = c // (NCORES // B), c % (NCORES // B)
        cols = slice(CW * j, CW * (j + 1))
        xt = np.ascontiguousarray(hs[b].T.astype(BF16))                      # [D, S]
        wq = np.ascontiguousarray(W_q[:, cols].reshape(DK, P, CW).transpose(1, 0, 2).astype(BF16))
        wk = np.ascontiguousarray(W_k[:, cols].reshape(DK, P, CW).transpose(1, 0, 2).astype(BF16))
        wv = np.ascontiguousarray(W_v[:, cols].reshape(DK, P, CW).transpose(1, 0, 2).astype(BF16))
        wo = np.ascontiguousarray(W_o[cols, :].reshape(CW // P, P, D).transpose(1, 0, 2))
        bqc = np.ascontiguousarray(b_q[cols].reshape(CW // P, P).T)          # [128, 2]
        bkc = np.ascontiguousarray(b_k[cols].reshape(CW // P, P).T)
        bvc = np.ascontiguousarray(b_v[cols].reshape(CW // P, P).T)         # [128, 2]
        m = mask[b * H + HPC * j: b * H + HPC * (j + 1), 0, :].astype(np.float32)  # [4, S]
        vm = np.ascontiguousarray(m.reshape(HPC, SP, P).transpose(2, 1, 0))  # [128, 16, 4]
        in_maps.append({
            "xt": xt, "wq": wq, "wk": wk, "wv": wv, "wo": wo,
            "bq": bqc, "bk": bkc, "bv": bvc, "vmask": vm,
        })
    return in_maps


def run(inputs, trace=False, **trace_kwargs):
    """Run the SPMD kernel. Returns (full_output, BassKernelResults)."""
    from concourse.bass_utils import run_bass_kernel_spmd

    nc = _get_nc()
    in_maps = _make_in_maps(
        inputs["hidden_states"], inputs["attention_mask"],
        inputs["W_q"], inputs["b_q"], inputs["W_k"], inputs["b_k"],
        inputs["W_v"], inputs["b_v"], inputs["W_o"])
    res = run_bass_kernel_spmd(
        nc, in_maps, list(range(NCORES)), trace=trace, **trace_kwargs)

    b_o = np.asarray(inputs["b_o"], dtype=np.float32)
    out = np.zeros((B, S, D), dtype=np.float32)
    gpb = NCORES // B
    for c in range(NCORES):
        out[c // gpb] += res.results[c]["y"]
    out += b_o[None, None, :]
    return out, res


def kernel(**inputs):
    out, _ = run(inputs, trace=False)
    return out



# revision 65
# speedup vs baseline: 1.1610x; 1.1610x over previous
"""BERT self-attention on 8 Trainium2 NeuronCores.

Sharding: data-parallel over batch (4 cores per batch element) x
tensor-parallel over heads (4 heads per core). Q/K/V projections are
column-sharded, the output projection is row-sharded; each core returns a
partial [S, D] output which the host sums (+ b_o).

Per-core math (batch b, heads hs = 4 heads, cols = 256 feature slice):
  QT = Wq_sl.T @ X_b.T        [256, 2048]   (bias via ScalarE Identity)
  KT = Wk_sl.T @ X_b.T        [256, 2048]   (m=0 half up front, m=1 in bg)
  V  = X_b @ Wv_sl            [2048, 256] -> V' = [V_h*mask | mask] per
                              head (b_v is folded into normalize instead:
                              exact since sum(probs) = 1 per query)
  per head h, q-block qb (512 wide):
    S^T tile [128k, 512q] = K_h @ Q_h^T slices  (PSUM)
    expS = exp(S^T / 8)                          (ACT, PSUM->SBUF)
    O'   = V'_h.T @ expS   accumulated over 16 k-tiles -> [65, 512]
           rows 0:64 = unnormalized O_h^T, row 64 = softmax denominator
  normalize: O^T = O^T * (1/den) + b_v (fast approx reciprocal; den
  broadcast across partitions via a stride-0 DMA read from DRAM staging)
  Y_partial = O^T.T @ Wo_sl   [2048, 1024]

Schedule: the exp stream on ScalarE is the attention-phase pacer, so
attention starts right after QT + KT(m=0); the KT m=1 chunks and all V
s-tiles are drained one-per-score-group as background PE work inside the
attention loop (sharing one PSUM bank that is later handed to the Y
projection), and PV groups are drained from a deep pending queue that
keeps the PE dense between exp handoffs. Bias adds run on ScalarE
(Identity+bias), the V'-mask fold is a single stride-0-broadcast
tensor-tensor multiply, and softmax division uses the single-pass
reciprocal_approx_fast.

Matmuls accumulate in fp32 PSUM; the QK/QT path and V'/expS run in bf16
(full PE rate + fast weight load), the output projection in float32r
(FP22). K tiles are zero-padded to the full 128 contraction rows because
the PE clock gate (HAM) only unthrottles for full-row matmuls.
"""

import sys

for _p in ("/root/.axon_site/_ro/trn_rl_repo", "/opt/trn_rl_repo"):
    if _p not in sys.path:
        sys.path.append(_p)

import numpy as np
import ml_dtypes

BF16 = ml_dtypes.bfloat16

B, S, D, H, DH = 2, 2048, 1024, 16, 64
P = 128
NCORES = 8
HPC = 4              # heads per core
CW = HPC * DH        # 256: per-core feature slice width
DK = D // P          # 8 k-tiles over the model dim
SP = S // P          # 16 s-tiles
NB = 4               # 512-wide blocks over S
NW = S // NB         # 512
G = 3                # exp kt-group size (PSUM banks per stage tile)

_STATE = {}


def _build_nc():
    import concourse.bacc as bacc
    import concourse.tile as tile
    from concourse import mybir

    f32 = mybir.dt.float32
    f32r = mybir.dt.float32r
    bf16 = mybir.dt.bfloat16
    Exp = mybir.ActivationFunctionType.Exp
    ActId = mybir.ActivationFunctionType.Identity

    nc = bacc.Bacc(None, target_bir_lowering=False, debug=False)

    with tile.TileContext(nc) as tc:
        with tc.tile_pool(name="dram", bufs=1, space="DRAM") as dram:
            xt = dram.tile([D, S], bf16, kind="ExternalInput", name="xt", uniquify=False)
            wq = dram.tile([P, DK, CW], bf16, kind="ExternalInput", name="wq", uniquify=False)
            wk = dram.tile([P, DK, CW], bf16, kind="ExternalInput", name="wk", uniquify=False)
            wv = dram.tile([P, DK, CW], bf16, kind="ExternalInput", name="wv", uniquify=False)
            wo = dram.tile([P, CW // P, D], f32, kind="ExternalInput", name="wo", uniquify=False)
            bq = dram.tile([P, CW // P], f32, kind="ExternalInput", name="bq", uniquify=False)
            bk = dram.tile([P, CW // P], f32, kind="ExternalInput", name="bk", uniquify=False)
            bv = dram.tile([P, CW // P], f32, kind="ExternalInput", name="bv", uniquify=False)
            vmask = dram.tile([P, SP, HPC], f32, kind="ExternalInput", name="vmask", uniquify=False)
            y = dram.tile([S, D], f32, kind="ExternalOutput", name="y", uniquify=False)
            dden = dram.tile([HPC * NB, NW], f32, name="dden")

            import concourse.bass as bass

            consts_cm = tc.tile_pool(name="consts", bufs=1)
            consts = consts_cm.__enter__()
            xt_sb = consts.tile([P, DK, S], bf16, name="xt_sb")
            wq_sb = consts.tile([P, DK, CW], bf16, name="wq_sb")
            wk_sb = consts.tile([P, DK, CW], bf16, name="wk_sb")
            wv_sb = consts.tile([P, DK, CW], bf16, name="wv_sb")
            wo_sb = consts.tile([P, CW // P, D], f32r, name="wo_sb")
            bq_sb = consts.tile([P, CW // P], f32, name="bq_sb")
            bk_sb = consts.tile([P, CW // P], f32, name="bk_sb")
            bv_sb = consts.tile([P, CW // P], f32, name="bv_sb")
            vmask_sb = consts.tile([P, SP, HPC], f32, name="vmask_sb")
            qt_sb = consts.tile([P, CW // P, S], bf16, name="qt_sb")
            kz_sb = consts.tile([P, HPC, SP, P], bf16, name="kz_sb")
            zsrc = consts.tile([P, NW], bf16, name="zsrc")
            ones_sb = consts.tile([P, DH], f32, name="ones_sb")
            vp_sb = consts.tile([P, SP, HPC, DH + 1], bf16, name="vp_sb")
            ot_sb = consts.tile([P, CW // P, S], f32r, name="ot_sb")

            # ---- input DMAs (small tensors first so the first matmuls
            # aren't queued behind the 8MB of X^T) ----
            nc.sync.dma_start(out=wq_sb[:], in_=wq[:])
            nc.sync.dma_start(out=bq_sb[:], in_=bq[:])
            for k in range(DK):
                for hlf in range(2):
                    hs_ = slice(hlf * (S // 2), (hlf + 1) * (S // 2))
                    nc.sync.dma_start(out=xt_sb[:, k, hs_],
                                      in_=xt[k * P:(k + 1) * P, hs_])
            nc.sync.dma_start(out=wk_sb[:], in_=wk[:])
            nc.sync.dma_start(out=bk_sb[:], in_=bk[:])
            nc.sync.dma_start(out=wv_sb[:], in_=wv[:])
            nc.sync.dma_start(out=wo_sb[:], in_=wo[:].bitcast(f32r))
            nc.sync.dma_start(out=bv_sb[:], in_=bv[:])
            nc.sync.dma_start(out=vmask_sb[:], in_=vmask[:])

            # zero-fill kz (stride-0 free-dim broadcast of a zeroed tile);
            # each head's K^T occupies its natural 64 partition rows, the
            # other 64 rows stay zero so the scores matmul contracts over
            # the full 128 partitions (HAM keeps the PE clock warm only
            # for full-row matmuls)
            nc.vector.memset(zsrc[:], 0.0)
            nc.vector.memset(ones_sb[:], 1.0)
            zview = zsrc[:]
            zbc = bass.AP(
                tensor=zview.tensor, offset=zview.offset,
                ap=[list(zview.ap[0]), [0, HPC * SP * P // NW]] + list(zview.ap[1:]))
            nc.vector.tensor_copy(out=kz_sb[:], in_=zbc)

            # warm-up burst: ~12 dummy matmuls on zeros while the X^T DMA
            # streams in, so the PE clock gate (HAM) is already at 8/8 when
            # the real projection matmuls start
            with tc.tile_pool(name="warm_psum", bufs=1, space="PSUM") as warm_psum:
                wps = warm_psum.tile([P, NW], f32, name="wps")
                for _ in range(12):
                    nc.tensor.matmul(wps[:], zsrc[:, 0:P], zsrc[:],
                                     start=True, stop=True)

            # ---- projections ----
            # QT first, k-outer (8 live PSUM accumulators) so matmuls pace
            # with the X^T DMA stream instead of waiting for all of it
            with tc.tile_pool(name="proj_psum", bufs=1, space="PSUM") as proj_psum:
                psqs = [proj_psum.tile([P, NB, NW], f32, name=f"psq{m}")
                        for m in range(CW // P)]
                for k in range(DK):
                    for m in range(CW // P):
                        for nb in range(NB):
                            nc.tensor.matmul(
                                psqs[m][:, nb, :],
                                wq_sb[:, k, m * P:(m + 1) * P],
                                xt_sb[:, k, nb * NW:(nb + 1) * NW],
                                start=(k == 0), stop=(k == DK - 1))
                # bias adds split across both free engines so the psq
                # banks release in parallel: m=0 per-nb on ScalarE (first
                # score block's qt slice ready earliest), m=1 fused on DVE
                for nb in range(NB):
                    nc.scalar.activation(
                        out=qt_sb[:, 0, nb * NW:(nb + 1) * NW],
                        in_=psqs[0][:, nb, :], func=ActId,
                        bias=bq_sb[:, 0:1])
                nc.vector.tensor_scalar_add(
                    out=qt_sb[:, 1, :].rearrange("p (a b) -> p a b", a=NB),
                    in0=psqs[1][:], scalar1=bq_sb[:, 1:2])

            # K projection: only the m=0 half (heads 0/1) before attention —
            # the m=1 half and the whole V projection are emitted as
            # background work inside the attention loop, filling the PE
            # while the exp stream (the attention-phase pacer) runs.
            def emit_kt_chunk(kpool, m, nb, on_act=True):
                psk = kpool.tile([P, NW], f32, name="bgp")
                for k in range(DK):
                    nc.tensor.matmul(
                        psk[:], wk_sb[:, k, m * P:(m + 1) * P],
                        xt_sb[:, k, nb * NW:(nb + 1) * NW],
                        start=(k == 0), stop=(k == DK - 1))
                # bias-scatter on ScalarE pre-attention (it is idle then),
                # but on DVE for background chunks inside the attention
                # loop — extra ACT work there delays the exp stream, the
                # attention-phase pacer
                if on_act:
                    nc.scalar.activation(
                        out=kz_sb[0:DH, 2 * m, nb * 4:(nb + 1) * 4, :],
                        in_=psk[0:DH, :].rearrange("p (a b) -> p a b", a=4),
                        func=ActId, bias=bk_sb[0:DH, m:m + 1])
                    nc.scalar.activation(
                        out=kz_sb[DH:P, 2 * m + 1, nb * 4:(nb + 1) * 4, :],
                        in_=psk[DH:P, :].rearrange("p (a b) -> p a b", a=4),
                        func=ActId, bias=bk_sb[DH:P, m:m + 1])
                else:
                    nc.vector.tensor_scalar_add(
                        out=kz_sb[0:DH, 2 * m, nb * 4:(nb + 1) * 4, :],
                        in0=psk[0:DH, :].rearrange("p (a b) -> p a b", a=4),
                        scalar1=bk_sb[0:DH, m:m + 1])
                    nc.vector.tensor_scalar_add(
                        out=kz_sb[DH:P, 2 * m + 1, nb * 4:(nb + 1) * 4, :],
                        in0=psk[DH:P, :].rearrange("p (a b) -> p a b", a=4),
                        scalar1=bk_sb[DH:P, m:m + 1])

            def emit_v_st(vpool, st):
                bgt = vpool.tile([P, NW], f32, name="bgp")
                psv = bgt[:, 0:CW]
                for k in range(DK):
                    nc.tensor.matmul(
                        psv, xt_sb[:, k, st * P:(st + 1) * P],
                        wv_sb[:, k, :],
                        start=(k == 0), stop=(k == DK - 1))
                # mask-fold + scatter into [h, 65]-strided V' slots, one
                # tensor_tensor with a stride-0 d-broadcast of the mask.
                # b_v is NOT added here: since sum(probs)=1 per query, the
                # V bias commutes with attention and is added during
                # normalize instead.
                vm = vmask_sb[:, st, :]
                vm_bc = bass.AP(
                    tensor=vm.tensor, offset=vm.offset,
                    ap=list(vm.ap) + [[0, DH]])
                nc.vector.tensor_mul(
                    out=vp_sb[:, st, :, 0:DH],
                    in0=psv.rearrange("p (h d) -> p h d", h=HPC),
                    in1=vm_bc)
                # ones column times mask == mask itself
                nc.vector.tensor_copy(
                    out=vp_sb[:, st, :, DH:DH + 1], in_=vmask_sb[:, st, :])

            with tc.tile_pool(name="kproj_psum", bufs=2, space="PSUM") as kproj_psum:
                for nb in range(NB):
                    emit_kt_chunk(kproj_psum, 0, nb)

            # ---- attention + inline normalize/output projection ----
            # qb-outer so each q-block's normalize + Y projection can be
            # emitted one head-iteration behind, overlapping the next
            # block's attention and keeping the PE dense (and HAM-warm)
            kt_groups = [list(range(g * G, min(g * G + G, SP))) for g in range((SP + G - 1) // G)]

            sb_pools_cm = [
                tc.tile_pool(name="exps_pool", bufs=15),
                tc.tile_pool(name="st65_pool", bufs=3),
                tc.tile_pool(name="bcast_pool", bufs=2),
                tc.tile_pool(name="y_pool", bufs=2),
            ]
            exps_pool, st65_pool, bcast_pool, y_pool = [c.__enter__() for c in sb_pools_cm]
            # background-work PSUM bank (KT m=1 chunks, V s-tiles) lives
            # until the bg queue drains, then its bank is handed to y_psum.
            # Pools must close in stack order, so bgpool enters last.
            bgpool_cm = tc.tile_pool(name="bg_psum", bufs=1, space="PSUM")
            y_psum_cm = tc.tile_pool(name="y_psum", bufs=1, space="PSUM")
            y_psum = None
            with tc.tile_pool(name="stage_psum", bufs=2, space="PSUM") as stage_psum, \
                 tc.tile_pool(name="op_psum", bufs=1, space="PSUM") as op_psum:
                bgpool = bgpool_cm.__enter__()

                def emit_pv(p):
                    op_p, h_p, kts_p, ex_p = p
                    for i, kt in enumerate(kts_p):
                        nc.tensor.matmul(
                            op_p[:], vp_sb[:, kt, h_p, :], ex_p[:, i, :],
                            start=(kt == 0), stop=(kt == SP - 1),
                            skip_group_check=True)

                tail_st65 = {}

                def emit_evict(p):
                    op_p, h_p, qb_p = p
                    qs = slice(qb_p * NW, (qb_p + 1) * NW)
                    mt, po = h_p // 2, (h_p % 2) * DH
                    st65 = st65_pool.tile([DH + 1, NW], f32, name="st65")
                    nc.vector.tensor_copy(out=st65[:], in_=op_p[:])
                    nc.sync.dma_start(
                        out=ot_sb[po:po + DH, mt, qs], in_=st65[0:DH, :].bitcast(f32r))
                    nc.sync.dma_start(
                        out=dden[h_p * NB + qb_p, :], in_=st65[DH:DH + 1, :])
                    if qb_p == NB - 1 and h_p >= 2:
                        # the tail normalize broadcasts these rows via a
                        # PE ones-matmul instead of a DRAM roundtrip
                        tail_st65[h_p] = st65

                def emit_normalize(qb_p, mt):
                    qs = slice(qb_p * NW, (qb_p + 1) * NW)
                    bc = bcast_pool.tile([P, NW], f32, name="bc")
                    for half in range(2):
                        hh = 2 * mt + half
                        den_row = dden[hh * NB + qb_p:hh * NB + qb_p + 1, :]
                        den_bcast = bass.AP(
                            tensor=den_row.tensor,
                            offset=den_row.offset,
                            ap=[[0, DH]] + list(den_row.ap[1:]),
                        )
                        nc.sync.dma_start(
                            out=bc[half * DH:(half + 1) * DH, :], in_=den_bcast)
                    nc.vector.reciprocal_approx_fast(out=bc[:], in_=bc[:])
                    # ot = ot * (1/den) + b_v  (b_v folded in here; exact
                    # because sum(probs) = 1 per query)
                    nc.vector.tensor_mul(
                        out=ot_sb[:, mt, qs],
                        in0=ot_sb[:, mt, qs], in1=bc[:])
                    nc.vector.tensor_scalar_add(
                        out=ot_sb[:, mt, qs],
                        in0=ot_sb[:, mt, qs], scalar1=bv_sb[:, mt:mt + 1])

                def emit_y_st(st):
                    yt = y_pool.tile([P, D], f32, name="yt")
                    for n2 in range(2):
                        yps = y_psum.tile([P, NW], f32, name="yps")
                        for k2 in range(CW // P):
                            nc.tensor.matmul(
                                yps[:], ot_sb[:, k2, st * P:(st + 1) * P],
                                wo_sb[:, k2, n2 * NW:(n2 + 1) * NW],
                                start=(k2 == 0), stop=(k2 == CW // P - 1))
                        nc.vector.tensor_copy(
                            out=yt[:, n2 * NW:(n2 + 1) * NW], in_=yps[:])
                    nc.sync.dma_start(out=y[st * P:(st + 1) * P, :], in_=yt[:])

                from collections import deque
                pends = deque()     # (op_ps, h, kts, ex, last, qb)
                evicted = set()
                def drain_one():
                    op_p, h_p, kts_p, ex_p, last, qb_p = pends.popleft()
                    emit_pv((op_p, h_p, kts_p, ex_p))
                    if last:
                        emit_evict((op_p, h_p, qb_p))
                        evicted.add((qb_p, h_p))

                def drain_until(qb_p, h_p):
                    while (qb_p, h_p) not in evicted:
                        drain_one()

                # background PE work drained one item per score-group: the
                # m=1 K chunks first (needed by h>=2), then the V s-tiles
                bg = deque([("kt", nb) for nb in range(NB)]
                           + [("v", st) for st in range(SP)])
                v_drained = 0

                for qb in range(NB):
                    qs = slice(qb * NW, (qb + 1) * NW)
                    for h in range(HPC):
                        mt = h // 2
                        if h == 0 and qb > 0:
                            drain_until(qb - 1, 1)
                            emit_normalize(qb - 1, 0)
                        if h == 1 and qb > 0:
                            drain_until(qb - 1, 3)
                            emit_normalize(qb - 1, 1)
                        op_ps = op_psum.tile([DH + 1, NW], f32, name="op_ps")
                        for gi, kts in enumerate(kt_groups):
                            ng = len(kts)
                            # emit the background/PV drain work BEFORE the
                            # score matmuls: the first score MM of a group
                            # waits on the exp drain of two groups back
                            # (stage-bank recycle), and with ACT saturated
                            # that semaphore arrives just-in-time — queuing
                            # independent PE work first overlaps the wait
                            # instead of idling at the FIFO head
                            if bg:
                                kind, idx = bg.popleft()
                                if kind == "kt":
                                    emit_kt_chunk(bgpool, 1, idx, on_act=False)
                                else:
                                    emit_v_st(bgpool, idx)
                                    v_drained += 1
                                # extra PV drain only when its V' s-tiles
                                # are already emitted (read-after-write on
                                # vp_sb must follow program order); when
                                # blocked, burn the slot on a second bg
                                # item so the queue unblocks sooner
                                if len(pends) >= 9:
                                    if not bg or max(pends[0][2]) < v_drained:
                                        drain_one()
                                    else:
                                        kind, idx = bg.popleft()
                                        if kind == "kt":
                                            emit_kt_chunk(bgpool, 1, idx,
                                                          on_act=False)
                                        else:
                                            emit_v_st(bgpool, idx)
                                            v_drained += 1
                                if not bg:
                                    # hand the bg bank to the Y projection
                                    bgpool_cm.__exit__(None, None, None)
                                    y_psum = y_psum_cm.__enter__()
                            else:
                                if pends:
                                    drain_one()
                                if len(pends) >= 8:
                                    drain_one()
                            if qb > 0 and h >= 2 and gi in (1, 3):
                                sts = 4 * (qb - 1) + 2 * (h - 2) + (gi - 1) // 2
                                emit_y_st(sts)
                            st_ps = stage_psum.tile([P, G, NW], f32, name="st_ps")
                            for i, kt in enumerate(kts):
                                nc.tensor.matmul(
                                    st_ps[:, i, :],
                                    kz_sb[:, h, kt, :],
                                    qt_sb[:, mt, qs],
                                    start=True, stop=True)
                            ex = exps_pool.tile([P, G, NW], bf16, name="ex")
                            nc.scalar.activation(
                                out=ex[:, 0:ng, :], in_=st_ps[:, 0:ng, :],
                                func=Exp, scale=1.0 / np.sqrt(DH))
                            pends.append((op_ps, h, kts, ex, gi == len(kt_groups) - 1, qb))
                # tail: overlap the last q-block's mt0 normalize and the
                # first half of the mt1 denominator broadcast with the
                # remaining PV drains
                drain_until(NB - 1, 1)
                emit_normalize(NB - 1, 0)
                drain_until(NB - 1, 2)
                # prefetch the h2 half of the last denominator broadcast so
                # only the h3 half remains on the tail critical path
                bct = bcast_pool.tile([P, NW], f32, name="bct")
                r0 = 2 * NB + (NB - 1)          # dden row for (head 2, qb3)
                den_row = dden[r0:r0 + 1, :]
                den_bcast = bass.AP(
                    tensor=den_row.tensor, offset=den_row.offset,
                    ap=[[0, DH]] + list(den_row.ap[1:]))
                nc.sync.dma_start(out=bct[0:DH, :], in_=den_bcast)
                while pends:
                    drain_one()
                if y_psum is not None:
                    y_psum_cm.__exit__(None, None, None)

            # tail: last q-block's remaining normalize + output projection,
            # with the attention PSUM banks freed for deeper Y buffering
            with tc.tile_pool(name="y2_psum", bufs=4, space="PSUM") as y2_psum:
                # last q-block, second head-pair: normalize per 128-wide
                # chunk so each Y st-tile starts as soon as its slice is
                # ready instead of waiting on the full 512-wide mul
                qb_t, mt_t = NB - 1, 1
                r1 = (2 * mt_t + 1) * NB + qb_t  # dden row for (head 3, qb3)
                den_row = dden[r1:r1 + 1, :]
                den_bcast = bass.AP(
                    tensor=den_row.tensor, offset=den_row.offset,
                    ap=[[0, DH]] + list(den_row.ap[1:]))
                nc.sync.dma_start(out=bct[DH:P, :], in_=den_bcast)
                nc.vector.reciprocal_approx_fast(out=bct[:], in_=bct[:])
                for st in range(4 * (NB - 1), 4 * NB):
                    cs = slice((st % 4) * P, (st % 4 + 1) * P)
                    qs = slice(st * P, (st + 1) * P)
                    nc.vector.tensor_mul(
                        out=ot_sb[:, mt_t, qs], in0=ot_sb[:, mt_t, qs],
                        in1=bct[:, cs])
                    nc.vector.tensor_scalar_add(
                        out=ot_sb[:, mt_t, qs], in0=ot_sb[:, mt_t, qs],
                        scalar1=bv_sb[:, mt_t:mt_t + 1])
                    yt = y_pool.tile([P, D], f32, name="yt")
                    for n2 in range(2):
                        yps = y2_psum.tile([P, NW], f32, name="yps")
                        for k2 in range(CW // P):
                            nc.tensor.matmul(
                                yps[:], ot_sb[:, k2, qs],
                                wo_sb[:, k2, n2 * NW:(n2 + 1) * NW],
                                start=(k2 == 0), stop=(k2 == CW // P - 1))
                        # PSUM->SBUF evictions split across ScalarE (idle
                        # after the last exp) and DVE so neither serializes
                        # the tail
                        if n2 == 0:
                            nc.scalar.activation(
                                out=yt[:, 0:NW], in_=yps[:], func=ActId)
                        else:
                            nc.vector.tensor_copy(
                                out=yt[:, NW:D], in_=yps[:])
                    nc.sync.dma_start(out=y[st * P:(st + 1) * P, :], in_=yt[:])

            for c in reversed(sb_pools_cm):
                c.__exit__(None, None, None)
            consts_cm.__exit__(None, None, None)

    nc.compile()
    return nc


def _get_nc():
    if "nc" not in _STATE:
        _STATE["nc"] = _build_nc()
    return _STATE["nc"]


def _make_in_maps(hidden_states, attention_mask, W_q, b_q, W_k, b_k, W_v, b_v, W_o):
    hs = np.asarray(hidden_states, dtype=np.float32)
    mask = np.asarray(attention_mask)
    W_q = np.asarray(W_q, dtype=np.float32)
    W_k = np.asarray(W_k, dtype=np.float32)
    W_v = np.asarray(W_v, dtype=np.float32)
    W_o = np.asarray(W_o, dtype=np.float32)
    b_q = np.asarray(b_q, dtype=np.float32)
    b_k = np.asarray(b_k, dtype=np.float32)
    b_v = np.asarray(b_v, dtype=np.float32)

    in_maps = []
    for c in range(NCORES):
        b, j = c // (NCORES // B), c % (NCORES // B)
        cols = slice(CW * j, CW * (j + 1))
        xt = np.ascontiguousarray(hs[b].T.astype(BF16))                      # [D, S]
        wq = np.ascontiguousarray(W_q[:, cols].reshape(DK, P, CW).transpose(1, 0, 2).astype(BF16))
        wk = np.ascontiguousarray(W_k[:, cols].reshape(DK, P, CW).transpose(1, 0, 2).astype(BF16))
        wv = np.ascontiguousarray(W_v[:, cols].reshape(DK, P, CW).transpose(1, 0, 2).astype(BF16))
        wo = np.ascontiguousarray(W_o[cols, :].reshape(CW // P, P, D).transpose(1, 0, 2))
        bqc = np.ascontiguousarray(b_q[cols].reshape(CW // P, P).T)          # [128, 2]
        bkc = np.ascontiguousarray(b_k[cols].reshape(CW // P, P).T)
        bvc = np.ascontiguousarray(b_v[cols].reshape(CW // P, P).T)         # [128, 2]
        m = mask[b * H + HPC * j: b * H + HPC * (j + 1), 0, :].astype(np.float32)  # [4, S]
        vm = np.ascontiguousarray(m.reshape(HPC, SP, P).transpose(2, 1, 0))  # [128, 16, 4]
        in_maps.append({
            "xt": xt, "wq": wq, "wk": wk, "wv": wv, "wo": wo,
            "bq": bqc, "bk": bkc, "bv": bvc, "vmask": vm,
        })
    return in_maps


def run(inputs, trace=False, **trace_kwargs):
    """Run the SPMD kernel. Returns (full_output, BassKernelResults)."""
    from concourse.bass_utils import run_bass_kernel_spmd

    nc = _get_nc()
    in_maps = _make_in_maps(
        inputs["hidden_states"], inputs["attention_mask"],
        inputs["W_q"], inputs["b_q"], inputs["W_k"], inputs["b_k"],
        inputs["W_v"], inputs["b_v"], inputs["W_o"])
    res = run_bass_kernel_spmd(
        nc, in_maps, list(range(NCORES)), trace=trace, **trace_kwargs)

    b_o = np.asarray(inputs["b_o"], dtype=np.float32)
    out = np.zeros((B, S, D), dtype=np.float32)
    gpb = NCORES // B
    for c in range(NCORES):
        out[c // gpb] += res.results[c]["y"]
    out += b_o[None, None, :]
    return out, res


def kernel(**inputs):
    out, _ = run(inputs, trace=False)
    return out



# revision 67
# speedup vs baseline: 1.1842x; 1.0200x over previous
"""BERT self-attention on 8 Trainium2 NeuronCores.

Sharding: data-parallel over batch (4 cores per batch element) x
tensor-parallel over heads (4 heads per core). Q/K/V projections are
column-sharded, the output projection is row-sharded; each core returns a
partial [S, D] output which the host sums (+ b_o).

Per-core math (batch b, heads hs = 4 heads, cols = 256 feature slice):
  QT = Wq_sl.T @ X_b.T        [256, 2048]   (bias via ScalarE Identity)
  KT = Wk_sl.T @ X_b.T        [256, 2048]   (m=0 half up front, m=1 in bg)
  V  = X_b @ Wv_sl            [2048, 256] -> V' = [V_h*mask | mask] per
                              head (b_v is folded into normalize instead:
                              exact since sum(probs) = 1 per query)
  per head h, q-block qb (512 wide):
    S^T tile [128k, 512q] = K_h @ Q_h^T slices  (PSUM)
    expS = exp(S^T / 8)                          (ACT, PSUM->SBUF)
    O'   = V'_h.T @ expS   accumulated over 16 k-tiles -> [65, 512]
           rows 0:64 = unnormalized O_h^T, row 64 = softmax denominator
  normalize: O^T = O^T * (1/den) + b_v (fast approx reciprocal; den
  broadcast across partitions via a stride-0 DMA read from DRAM staging)
  Y_partial = O^T.T @ Wo_sl   [2048, 1024]

Schedule: the exp stream on ScalarE is the attention-phase pacer, so
attention starts right after QT + KT(m=0); the KT m=1 chunks and all V
s-tiles are drained one-per-score-group as background PE work inside the
attention loop (sharing one PSUM bank that is later handed to the Y
projection), and PV groups are drained from a deep pending queue that
keeps the PE dense between exp handoffs. Bias adds run on ScalarE
(Identity+bias), the V'-mask fold is a single stride-0-broadcast
tensor-tensor multiply, and softmax division uses the single-pass
reciprocal_approx_fast.

Matmuls accumulate in fp32 PSUM; the QK/QT path and V'/expS run in bf16
(full PE rate + fast weight load), the output projection in float32r
(FP22). K tiles are zero-padded to the full 128 contraction rows because
the PE clock gate (HAM) only unthrottles for full-row matmuls.
"""

import sys

for _p in ("/root/.axon_site/_ro/trn_rl_repo", "/opt/trn_rl_repo"):
    if _p not in sys.path:
        sys.path.append(_p)

import numpy as np
import ml_dtypes

BF16 = ml_dtypes.bfloat16

B, S, D, H, DH = 2, 2048, 1024, 16, 64
P = 128
NCORES = 8
HPC = 4              # heads per core
CW = HPC * DH        # 256: per-core feature slice width
DK = D // P          # 8 k-tiles over the model dim
SP = S // P          # 16 s-tiles
NB = 4               # 512-wide blocks over S
NW = S // NB         # 512
G = 3                # exp kt-group size (PSUM banks per stage tile)

_STATE = {}


def _build_nc():
    import concourse.bacc as bacc
    import concourse.tile as tile
    from concourse import mybir

    f32 = mybir.dt.float32
    f32r = mybir.dt.float32r
    bf16 = mybir.dt.bfloat16
    Exp = mybir.ActivationFunctionType.Exp
    ActId = mybir.ActivationFunctionType.Identity

    nc = bacc.Bacc(None, target_bir_lowering=False, debug=False)

    with tile.TileContext(nc) as tc:
        with tc.tile_pool(name="dram", bufs=1, space="DRAM") as dram:
            xt = dram.tile([D, S], bf16, kind="ExternalInput", name="xt", uniquify=False)
            wq = dram.tile([P, DK, CW], bf16, kind="ExternalInput", name="wq", uniquify=False)
            wk = dram.tile([P, DK, CW], bf16, kind="ExternalInput", name="wk", uniquify=False)
            wv = dram.tile([P, DK, CW], bf16, kind="ExternalInput", name="wv", uniquify=False)
            wo = dram.tile([P, CW // P, D], f32, kind="ExternalInput", name="wo", uniquify=False)
            bq = dram.tile([P, CW // P], f32, kind="ExternalInput", name="bq", uniquify=False)
            bk = dram.tile([P, CW // P], f32, kind="ExternalInput", name="bk", uniquify=False)
            bv = dram.tile([P, CW // P], f32, kind="ExternalInput", name="bv", uniquify=False)
            vmask = dram.tile([P, SP, HPC], f32, kind="ExternalInput", name="vmask", uniquify=False)
            y = dram.tile([S, D], f32, kind="ExternalOutput", name="y", uniquify=False)
            dden = dram.tile([HPC * NB, NW], f32, name="dden")

            import concourse.bass as bass

            consts_cm = tc.tile_pool(name="consts", bufs=1)
            consts = consts_cm.__enter__()
            xt_sb = consts.tile([P, DK, S], bf16, name="xt_sb")
            wq_sb = consts.tile([P, DK, CW], bf16, name="wq_sb")
            wk_sb = consts.tile([P, DK, CW], bf16, name="wk_sb")
            wv_sb = consts.tile([P, DK, CW], bf16, name="wv_sb")
            wo_sb = consts.tile([P, CW // P, D], f32r, name="wo_sb")
            bq_sb = consts.tile([P, CW // P], f32, name="bq_sb")
            bk_sb = consts.tile([P, CW // P], f32, name="bk_sb")
            bv_sb = consts.tile([P, CW // P], f32, name="bv_sb")
            vmask_sb = consts.tile([P, SP, HPC], f32, name="vmask_sb")
            qt_sb = consts.tile([P, CW // P, S], bf16, name="qt_sb")
            kz_sb = consts.tile([P, HPC, SP, P], bf16, name="kz_sb")
            zsrc = consts.tile([P, NW], bf16, name="zsrc")
            ones_sb = consts.tile([P, DH], f32, name="ones_sb")
            vp_sb = consts.tile([P, SP, HPC, DH + 1], bf16, name="vp_sb")
            ot_sb = consts.tile([P, CW // P, S], f32r, name="ot_sb")

            # ---- input DMAs (small tensors first so the first matmuls
            # aren't queued behind the 8MB of X^T) ----
            nc.sync.dma_start(out=wq_sb[:], in_=wq[:])
            nc.sync.dma_start(out=bq_sb[:], in_=bq[:])
            for k in range(DK):
                for hlf in range(2):
                    hs_ = slice(hlf * (S // 2), (hlf + 1) * (S // 2))
                    nc.sync.dma_start(out=xt_sb[:, k, hs_],
                                      in_=xt[k * P:(k + 1) * P, hs_])
            nc.sync.dma_start(out=wk_sb[:], in_=wk[:])
            nc.sync.dma_start(out=bk_sb[:], in_=bk[:])
            nc.sync.dma_start(out=wv_sb[:], in_=wv[:])
            nc.sync.dma_start(out=wo_sb[:], in_=wo[:].bitcast(f32r))
            nc.sync.dma_start(out=bv_sb[:], in_=bv[:])
            nc.sync.dma_start(out=vmask_sb[:], in_=vmask[:])

            # zero-fill kz (stride-0 free-dim broadcast of a zeroed tile);
            # each head's K^T occupies its natural 64 partition rows, the
            # other 64 rows stay zero so the scores matmul contracts over
            # the full 128 partitions (HAM keeps the PE clock warm only
            # for full-row matmuls)
            nc.vector.memset(zsrc[:], 0.0)
            nc.vector.memset(ones_sb[:], 1.0)
            zview = zsrc[:]
            zbc = bass.AP(
                tensor=zview.tensor, offset=zview.offset,
                ap=[list(zview.ap[0]), [0, HPC * SP * P // NW]] + list(zview.ap[1:]))
            nc.vector.tensor_copy(out=kz_sb[:], in_=zbc)

            # warm-up burst: ~12 dummy matmuls on zeros while the X^T DMA
            # streams in, so the PE clock gate (HAM) is already at 8/8 when
            # the real projection matmuls start
            with tc.tile_pool(name="warm_psum", bufs=1, space="PSUM") as warm_psum:
                wps = warm_psum.tile([P, NW], f32, name="wps")
                for _ in range(12):
                    nc.tensor.matmul(wps[:], zsrc[:, 0:P], zsrc[:],
                                     start=True, stop=True)

            # ---- projections ----
            # QT first, k-outer (8 live PSUM accumulators) so matmuls pace
            # with the X^T DMA stream instead of waiting for all of it
            with tc.tile_pool(name="proj_psum", bufs=1, space="PSUM") as proj_psum:
                psqs = [proj_psum.tile([P, NB, NW], f32, name=f"psq{m}")
                        for m in range(CW // P)]
                for k in range(DK):
                    for m in range(CW // P):
                        for nb in range(NB):
                            nc.tensor.matmul(
                                psqs[m][:, nb, :],
                                wq_sb[:, k, m * P:(m + 1) * P],
                                xt_sb[:, k, nb * NW:(nb + 1) * NW],
                                start=(k == 0), stop=(k == DK - 1))
                # bias adds split across both free engines so the psq
                # banks release in parallel: m=0 per-nb on ScalarE (first
                # score block's qt slice ready earliest), m=1 fused on DVE
                for nb in range(NB):
                    nc.scalar.activation(
                        out=qt_sb[:, 0, nb * NW:(nb + 1) * NW],
                        in_=psqs[0][:, nb, :], func=ActId,
                        bias=bq_sb[:, 0:1])
                nc.vector.tensor_scalar_add(
                    out=qt_sb[:, 1, :].rearrange("p (a b) -> p a b", a=NB),
                    in0=psqs[1][:], scalar1=bq_sb[:, 1:2])

            # K projection: only the m=0 half (heads 0/1) before attention —
            # the m=1 half and the whole V projection are emitted as
            # background work inside the attention loop, filling the PE
            # while the exp stream (the attention-phase pacer) runs.
            def emit_kt_chunk(kpool, m, nb, on_act=True):
                psk = kpool.tile([P, NW], f32, name="bgp")
                for k in range(DK):
                    nc.tensor.matmul(
                        psk[:], wk_sb[:, k, m * P:(m + 1) * P],
                        xt_sb[:, k, nb * NW:(nb + 1) * NW],
                        start=(k == 0), stop=(k == DK - 1))
                # bias-scatter on ScalarE pre-attention (it is idle then),
                # but on DVE for background chunks inside the attention
                # loop — extra ACT work there delays the exp stream, the
                # attention-phase pacer
                if on_act:
                    nc.scalar.activation(
                        out=kz_sb[0:DH, 2 * m, nb * 4:(nb + 1) * 4, :],
                        in_=psk[0:DH, :].rearrange("p (a b) -> p a b", a=4),
                        func=ActId, bias=bk_sb[0:DH, m:m + 1])
                    nc.scalar.activation(
                        out=kz_sb[DH:P, 2 * m + 1, nb * 4:(nb + 1) * 4, :],
                        in_=psk[DH:P, :].rearrange("p (a b) -> p a b", a=4),
                        func=ActId, bias=bk_sb[DH:P, m:m + 1])
                else:
                    nc.vector.tensor_scalar_add(
                        out=kz_sb[0:DH, 2 * m, nb * 4:(nb + 1) * 4, :],
                        in0=psk[0:DH, :].rearrange("p (a b) -> p a b", a=4),
                        scalar1=bk_sb[0:DH, m:m + 1])
                    nc.vector.tensor_scalar_add(
                        out=kz_sb[DH:P, 2 * m + 1, nb * 4:(nb + 1) * 4, :],
                        in0=psk[DH:P, :].rearrange("p (a b) -> p a b", a=4),
                        scalar1=bk_sb[DH:P, m:m + 1])

            def emit_v_st(vpool, st):
                bgt = vpool.tile([P, NW], f32, name="bgp")
                psv = bgt[:, 0:CW]
                for k in range(DK):
                    nc.tensor.matmul(
                        psv, xt_sb[:, k, st * P:(st + 1) * P],
                        wv_sb[:, k, :],
                        start=(k == 0), stop=(k == DK - 1))
                # mask-fold + scatter into [h, 65]-strided V' slots, one
                # tensor_tensor with a stride-0 d-broadcast of the mask.
                # b_v is NOT added here: since sum(probs)=1 per query, the
                # V bias commutes with attention and is added during
                # normalize instead.
                vm = vmask_sb[:, st, :]
                vm_bc = bass.AP(
                    tensor=vm.tensor, offset=vm.offset,
                    ap=list(vm.ap) + [[0, DH]])
                nc.vector.tensor_mul(
                    out=vp_sb[:, st, :, 0:DH],
                    in0=psv.rearrange("p (h d) -> p h d", h=HPC),
                    in1=vm_bc)
                # ones column times mask == mask itself
                nc.vector.tensor_copy(
                    out=vp_sb[:, st, :, DH:DH + 1], in_=vmask_sb[:, st, :])

            with tc.tile_pool(name="kproj_psum", bufs=2, space="PSUM") as kproj_psum:
                for nb in range(NB):
                    emit_kt_chunk(kproj_psum, 0, nb)

            # ---- attention + inline normalize/output projection ----
            # qb-outer so each q-block's normalize + Y projection can be
            # emitted one head-iteration behind, overlapping the next
            # block's attention and keeping the PE dense (and HAM-warm)
            kt_groups = [list(range(g * G, min(g * G + G, SP))) for g in range((SP + G - 1) // G)]

            sb_pools_cm = [
                tc.tile_pool(name="exps_pool", bufs=15),
                tc.tile_pool(name="st65_pool", bufs=3),
                tc.tile_pool(name="bcast_pool", bufs=2),
                tc.tile_pool(name="y_pool", bufs=2),
            ]
            exps_pool, st65_pool, bcast_pool, y_pool = [c.__enter__() for c in sb_pools_cm]
            # background-work PSUM bank (KT m=1 chunks, V s-tiles) lives
            # until the bg queue drains, then its bank is handed to y_psum.
            # Pools must close in stack order, so bgpool enters last.
            bgpool_cm = tc.tile_pool(name="bg_psum", bufs=1, space="PSUM")
            y_psum_cm = tc.tile_pool(name="y_psum", bufs=1, space="PSUM")
            y_psum = None
            with tc.tile_pool(name="stage_psum", bufs=2, space="PSUM") as stage_psum, \
                 tc.tile_pool(name="op_psum", bufs=1, space="PSUM") as op_psum:
                bgpool = bgpool_cm.__enter__()

                def emit_pv(p):
                    op_p, h_p, kts_p, ex_p = p
                    for i, kt in enumerate(kts_p):
                        nc.tensor.matmul(
                            op_p[:], vp_sb[:, kt, h_p, :], ex_p[:, i, :],
                            start=(kt == 0), stop=(kt == SP - 1),
                            skip_group_check=True)

                tail_st65 = {}

                def emit_evict(p):
                    op_p, h_p, qb_p = p
                    qs = slice(qb_p * NW, (qb_p + 1) * NW)
                    mt, po = h_p // 2, (h_p % 2) * DH
                    st65 = st65_pool.tile([DH + 1, NW], f32, name="st65")
                    nc.vector.tensor_copy(out=st65[:], in_=op_p[:])
                    nc.sync.dma_start(
                        out=ot_sb[po:po + DH, mt, qs], in_=st65[0:DH, :].bitcast(f32r))
                    nc.sync.dma_start(
                        out=dden[h_p * NB + qb_p, :], in_=st65[DH:DH + 1, :])
                    if qb_p == NB - 1 and h_p >= 2:
                        # the tail normalize broadcasts these rows via a
                        # PE ones-matmul instead of a DRAM roundtrip
                        tail_st65[h_p] = st65

                def emit_normalize(qb_p, mt):
                    qs = slice(qb_p * NW, (qb_p + 1) * NW)
                    bc = bcast_pool.tile([P, NW], f32, name="bc")
                    for half in range(2):
                        hh = 2 * mt + half
                        den_row = dden[hh * NB + qb_p:hh * NB + qb_p + 1, :]
                        den_bcast = bass.AP(
                            tensor=den_row.tensor,
                            offset=den_row.offset,
                            ap=[[0, DH]] + list(den_row.ap[1:]),
                        )
                        nc.sync.dma_start(
                            out=bc[half * DH:(half + 1) * DH, :], in_=den_bcast)
                    nc.vector.reciprocal_approx_fast(out=bc[:], in_=bc[:])
                    # ot = ot * (1/den) + b_v  (b_v folded in here; exact
                    # because sum(probs) = 1 per query)
                    nc.vector.tensor_mul(
                        out=ot_sb[:, mt, qs],
                        in0=ot_sb[:, mt, qs], in1=bc[:])
                    nc.vector.tensor_scalar_add(
                        out=ot_sb[:, mt, qs],
                        in0=ot_sb[:, mt, qs], scalar1=bv_sb[:, mt:mt + 1])

                def emit_y_st(st):
                    yt = y_pool.tile([P, D], f32, name="yt")
                    for n2 in range(2):
                        yps = y_psum.tile([P, NW], f32, name="yps")
                        for k2 in range(CW // P):
                            nc.tensor.matmul(
                                yps[:], ot_sb[:, k2, st * P:(st + 1) * P],
                                wo_sb[:, k2, n2 * NW:(n2 + 1) * NW],
                                start=(k2 == 0), stop=(k2 == CW // P - 1))
                        nc.vector.tensor_copy(
                            out=yt[:, n2 * NW:(n2 + 1) * NW], in_=yps[:])
                    nc.sync.dma_start(out=y[st * P:(st + 1) * P, :], in_=yt[:])

                from collections import deque
                pends = deque()     # (op_ps, h, kts, ex, last, qb)
                evicted = set()
                def drain_one():
                    op_p, h_p, kts_p, ex_p, last, qb_p = pends.popleft()
                    emit_pv((op_p, h_p, kts_p, ex_p))
                    if last:
                        emit_evict((op_p, h_p, qb_p))
                        evicted.add((qb_p, h_p))

                def drain_until(qb_p, h_p):
                    while (qb_p, h_p) not in evicted:
                        drain_one()

                # background PE work drained one item per score-group: the
                # m=1 K chunks early (needed by h>=2), interleaved with V
                # s-tiles so consecutive KT chunks don't serialize on the
                # single bg bank (each chunk's DVE scatter overlaps the
                # next V tile's matmuls instead of blocking the bank)
                bg = deque()
                for nb in range(NB):
                    bg.append(("kt", nb))
                    bg.append(("v", nb))
                bg.extend(("v", st) for st in range(NB, SP))
                v_drained = 0

                for qb in range(NB):
                    qs = slice(qb * NW, (qb + 1) * NW)
                    for h in range(HPC):
                        mt = h // 2
                        if h == 0 and qb > 0:
                            drain_until(qb - 1, 1)
                            emit_normalize(qb - 1, 0)
                        if h == 1 and qb > 0:
                            drain_until(qb - 1, 3)
                            emit_normalize(qb - 1, 1)
                        op_ps = op_psum.tile([DH + 1, NW], f32, name="op_ps")
                        for gi, kts in enumerate(kt_groups):
                            ng = len(kts)
                            st_ps = stage_psum.tile([P, G, NW], f32, name="st_ps")
                            for i, kt in enumerate(kts):
                                nc.tensor.matmul(
                                    st_ps[:, i, :],
                                    kz_sb[:, h, kt, :],
                                    qt_sb[:, mt, qs],
                                    start=True, stop=True)
                            if bg:
                                kind, idx = bg.popleft()
                                if kind == "kt":
                                    emit_kt_chunk(bgpool, 1, idx, on_act=False)
                                else:
                                    emit_v_st(bgpool, idx)
                                    v_drained += 1
                                # extra PV drain only when its V' s-tiles
                                # are already emitted (read-after-write on
                                # vp_sb must follow program order); when
                                # blocked, burn the slot on a second bg
                                # item so the queue unblocks sooner
                                if len(pends) >= 9:
                                    if not bg or max(pends[0][2]) < v_drained:
                                        drain_one()
                                    else:
                                        kind, idx = bg.popleft()
                                        if kind == "kt":
                                            emit_kt_chunk(bgpool, 1, idx,
                                                          on_act=False)
                                        else:
                                            emit_v_st(bgpool, idx)
                                            v_drained += 1
                                if not bg:
                                    # hand the bg bank to the Y projection
                                    bgpool_cm.__exit__(None, None, None)
                                    y_psum = y_psum_cm.__enter__()
                            else:
                                if pends:
                                    drain_one()
                                if len(pends) >= 8:
                                    drain_one()
                            if qb > 0 and h >= 2 and gi in (1, 3):
                                sts = 4 * (qb - 1) + 2 * (h - 2) + (gi - 1) // 2
                                emit_y_st(sts)
                            ex = exps_pool.tile([P, G, NW], bf16, name="ex")
                            nc.scalar.activation(
                                out=ex[:, 0:ng, :], in_=st_ps[:, 0:ng, :],
                                func=Exp, scale=1.0 / np.sqrt(DH))
                            pends.append((op_ps, h, kts, ex, gi == len(kt_groups) - 1, qb))
                # tail: overlap the last q-block's mt0 normalize and the
                # first half of the mt1 denominator broadcast with the
                # remaining PV drains
                drain_until(NB - 1, 1)
                emit_normalize(NB - 1, 0)
                drain_until(NB - 1, 2)
                # prefetch the h2 half of the last denominator broadcast so
                # only the h3 half remains on the tail critical path
                bct = bcast_pool.tile([P, NW], f32, name="bct")
                r0 = 2 * NB + (NB - 1)          # dden row for (head 2, qb3)
                den_row = dden[r0:r0 + 1, :]
                den_bcast = bass.AP(
                    tensor=den_row.tensor, offset=den_row.offset,
                    ap=[[0, DH]] + list(den_row.ap[1:]))
                nc.sync.dma_start(out=bct[0:DH, :], in_=den_bcast)
                while pends:
                    drain_one()
                if y_psum is not None:
                    y_psum_cm.__exit__(None, None, None)

            # tail: last q-block's remaining normalize + output projection,
            # with the attention PSUM banks freed for deeper Y buffering
            with tc.tile_pool(name="y2_psum", bufs=4, space="PSUM") as y2_psum:
                # last q-block, second head-pair: normalize per 128-wide
                # chunk so each Y st-tile starts as soon as its slice is
                # ready instead of waiting on the full 512-wide mul
                qb_t, mt_t = NB - 1, 1
                r1 = (2 * mt_t + 1) * NB + qb_t  # dden row for (head 3, qb3)
                den_row = dden[r1:r1 + 1, :]
                den_bcast = bass.AP(
                    tensor=den_row.tensor, offset=den_row.offset,
                    ap=[[0, DH]] + list(den_row.ap[1:]))
                nc.sync.dma_start(out=bct[DH:P, :], in_=den_bcast)
                nc.vector.reciprocal_approx_fast(out=bct[:], in_=bct[:])
                for st in range(4 * (NB - 1), 4 * NB):
                    cs = slice((st % 4) * P, (st % 4 + 1) * P)
                    qs = slice(st * P, (st + 1) * P)
                    nc.vector.tensor_mul(
                        out=ot_sb[:, mt_t, qs], in0=ot_sb[:, mt_t, qs],
                        in1=bct[:, cs])
                    nc.vector.tensor_scalar_add(
                        out=ot_sb[:, mt_t, qs], in0=ot_sb[:, mt_t, qs],
                        scalar1=bv_sb[:, mt_t:mt_t + 1])
                    yt = y_pool.tile([P, D], f32, name="yt")
                    for n2 in range(2):
                        yps = y2_psum.tile([P, NW], f32, name="yps")
                        for k2 in range(CW // P):
                            nc.tensor.matmul(
                                yps[:], ot_sb[:, k2, qs],
                                wo_sb[:, k2, n2 * NW:(n2 + 1) * NW],
                                start=(k2 == 0), stop=(k2 == CW // P - 1))
                        # PSUM->SBUF evictions split across ScalarE (idle
                        # after the last exp) and DVE so neither serializes
                        # the tail
                        if n2 == 0:
                            nc.scalar.activation(
                                out=yt[:, 0:NW], in_=yps[:], func=ActId)
                        else:
                            nc.vector.tensor_copy(
                                out=yt[:, NW:D], in_=yps[:])
                    nc.sync.dma_start(out=y[st * P:(st + 1) * P, :], in_=yt[:])

            for c in reversed(sb_pools_cm):
                c.__exit__(None, None, None)
            consts_cm.__exit__(None, None, None)

    nc.compile()
    return nc


def _get_nc():
    if "nc" not in _STATE:
        _STATE["nc"] = _build_nc()
    return _STATE["nc"]


def _make_in_maps(hidden_states, attention_mask, W_q, b_q, W_k, b_k, W_v, b_v, W_o):
    hs = np.asarray(hidden_states, dtype=np.float32)
    mask = np.asarray(attention_mask)
    W_q = np.asarray(W_q, dtype=np.float32)
    W_k = np.asarray(W_k, dtype=np.float32)
    W_v = np.asarray(W_v, dtype=np.float32)
    W_o = np.asarray(W_o, dtype=np.float32)
    b_q = np.asarray(b_q, dtype=np.float32)
    b_k = np.asarray(b_k, dtype=np.float32)
    b_v = np.asarray(b_v, dtype=np.float32)

    in_maps = []
    for c in range(NCORES):
        b, j = c // (NCORES // B), c % (NCORES // B)
        cols = slice(CW * j, CW * (j + 1))
        xt = np.ascontiguousarray(hs[b].T.astype(BF16))                      # [D, S]
        wq = np.ascontiguousarray(W_q[:, cols].reshape(DK, P, CW).transpose(1, 0, 2).astype(BF16))
        wk = np.ascontiguousarray(W_k[:, cols].reshape(DK, P, CW).transpose(1, 0, 2).astype(BF16))
        wv = np.ascontiguousarray(W_v[:, cols].reshape(DK, P, CW).transpose(1, 0, 2).astype(BF16))
        wo = np.ascontiguousarray(W_o[cols, :].reshape(CW // P, P, D).transpose(1, 0, 2))
        bqc = np.ascontiguousarray(b_q[cols].reshape(CW // P, P).T)          # [128, 2]
        bkc = np.ascontiguousarray(b_k[cols].reshape(CW // P, P).T)
        bvc = np.ascontiguousarray(b_v[cols].reshape(CW // P, P).T)         # [128, 2]
        m = mask[b * H + HPC * j: b * H + HPC * (j + 1), 0, :].astype(np.float32)  # [4, S]
        vm = np.ascontiguousarray(m.reshape(HPC, SP, P).transpose(2, 1, 0))  # [128, 16, 4]
        in_maps.append({
            "xt": xt, "wq": wq, "wk": wk, "wv": wv, "wo": wo,
            "bq": bqc, "bk": bkc, "bv": bvc, "vmask": vm,
        })
    return in_maps


def run(inputs, trace=False, **trace_kwargs):
    """Run the SPMD kernel. Returns (full_output, BassKernelResults)."""
    from concourse.bass_utils import run_bass_kernel_spmd

    nc = _get_nc()
    in_maps = _make_in_maps(
        inputs["hidden_states"], inputs["attention_mask"],
        inputs["W_q"], inputs["b_q"], inputs["W_k"], inputs["b_k"],
        inputs["W_v"], inputs["b_v"], inputs["W_o"])
    res = run_bass_kernel_spmd(
        nc, in_maps, list(range(NCORES)), trace=trace, **trace_kwargs)

    b_o = np.asarray(inputs["b_o"], dtype=np.float32)
    out = np.zeros((B, S, D), dtype=np.float32)
    gpb = NCORES // B
    for c in range(NCORES):
        out[c // gpb] += res.results[c]["y"]
    out += b_o[None, None, :]
    return out, res


def kernel(**inputs):
    out, _ = run(inputs, trace=False)
    return out

